# revision 31
# baseline (speedup 1.0000x reference)
"""Trainium2 Bass kernel for nn_MultiHeadAttention_73212012528234.

Sharding: data-parallel over batch — core c handles batch b=c (B=8, 8 cores).

Per-core pipeline (all matmuls bf16 on PE, fp32 PSUM accumulation):
  1. QT/KT = (W/8 or W) @ X^T  -> [D, S] head-major layout (d on partitions)
     V = X @ W_V^T             -> [S, D] natural layout
  2. per head h: scores psum = QT_h-slice x KT_h  (+ identity-matmul adds
     -1e9*mask);  E1 = exp(psum) with ACT accum -> rowsum r1
  3. tmid = (E1 * 1/r1) + bias2'  (DVE scalar_tensor_tensor);
     E2 = exp(w0 * tmid) with accum -> r2;  attn = E2 * 1/r2 -> HBM
     (bf16, host casts to f32)
  4. PE-transpose attn -> attnT;  contextT_h = V_h-slice^T x attnT
  5. outputT = W_fc^T-chunks x contextT -> HBM (f32, host transposes)

bias2' = (w1*exp(-dist) + w2*adj + cb - 1e9*mask)/w0 built on device; the /w0
fold lets exp2's ACT scale apply w0 with zero extra ops.
"""

from contextlib import ExitStack

import ml_dtypes
import numpy as np

import concourse.bass as bass
import concourse.tile as tile
from concourse import bacc, mybir
from concourse.bass import ts
from concourse.bass_utils import run_bass_kernel_spmd
from concourse.masks import make_identity

B, S, D, H, DK = 8, 512, 1024, 16, 64
P = 128
NQ = S // P  # 4 q/k chunks of 128
ND = D // P  # 8 d chunks of 128
BF16 = mybir.dt.bfloat16
F32 = mybir.dt.float32
Exp = mybir.ActivationFunctionType.Exp
Copy = mybir.ActivationFunctionType.Copy
ALU = mybir.AluOpType

# toggles (test.py may flip before calling kernel)
PROFILE = False
TRACE_KWARGS = {}
LAST_RESULTS = None


def _install_profile_hook():
    """The agent image's antenv lacks axon_hooks; synthesize it so
    run_bass_kernel_spmd(trace=True) can capture NTFF profiles."""
    import sys, types
    try:
        from antenv.axon_hooks import get_axon_ntff_profile_hook  # noqa
        return
    except ImportError:
        pass
    import antenv
    from trn_agent_boot.trn_boot import _ntff_profile_via_ctypes
    mod = types.ModuleType("antenv.axon_hooks")
    mod._hook = _ntff_profile_via_ctypes("/opt/axon/libaxon_pjrt.so")

    def set_axon_ntff_profile_hook(h):
        mod._hook = h

    def get_axon_ntff_profile_hook():
        return mod._hook

    mod.set_axon_ntff_profile_hook = set_axon_ntff_profile_hook
    mod.get_axon_ntff_profile_hook = get_axon_ntff_profile_hook
    sys.modules["antenv.axon_hooks"] = mod
    antenv.axon_hooks = mod


def _proj_T(nc, psum, w_tiles, xT_tiles, out_tiles, copy_engine):
    """out[o,s] = sum_d w[d,o-slice] * xT[d,s]  -> out_tiles: ND x [P, S]."""
    for o in range(ND):
        ps = psum.tile([P, S], F32, tag="proj", name=f"ps_{out_tiles[o].name}")
        for d in range(ND):
            nc.tensor.matmul(
                ps, lhsT=w_tiles[d][:, ts(o, P)], rhs=xT_tiles[d],
                start=(d == 0), stop=(d == ND - 1),
            )
        copy_engine(out=out_tiles[o], in_=ps)


def _build_program(w0, w1, w2, cb):
    fold = abs(w0) >= 1e-4
    iw0 = (1.0 / w0) if fold else 1.0
    exp2_scale = w0 if fold else 1.0

    nc = bacc.Bacc("TRN2", target_bir_lowering=False, debug=False)

    # inputs arrive partition-interleaved: t[p, d, c] = orig[d*128+p, c]
    xqT_d = nc.dram_tensor("xqT", [P, ND, S], BF16, kind="ExternalInput").ap()
    xkT_d = nc.dram_tensor("xkT", [P, ND, S], BF16, kind="ExternalInput").ap()
    xvT_d = nc.dram_tensor("xvT", [P, ND, S], BF16, kind="ExternalInput").ap()
    wqT_d = nc.dram_tensor("wqT", [P, ND, D], BF16, kind="ExternalInput").ap()
    wkT_d = nc.dram_tensor("wkT", [P, ND, D], BF16, kind="ExternalInput").ap()
    wvT_d = nc.dram_tensor("wvT", [P, ND, D], BF16, kind="ExternalInput").ap()
    wfT_d = nc.dram_tensor("wfT", [P, ND, D], BF16, kind="ExternalInput").ap()
    maskneg_d = nc.dram_tensor("maskneg", [P, NQ, S], BF16, kind="ExternalInput").ap()
    adj_d = nc.dram_tensor("adj", [P, NQ, S], BF16, kind="ExternalInput").ap()
    dist_d = nc.dram_tensor("dist", [P, NQ, S], BF16, kind="ExternalInput").ap()

    attn_d = [nc.dram_tensor(f"attn{h}", [P, NQ, S], BF16, kind="ExternalOutput").ap()
              for h in range(H)]
    outT_d = [nc.dram_tensor(f"outT{o}", [P, S], F32, kind="ExternalOutput").ap()
              for o in range(ND)]

    with tile.TileContext(nc) as tc, ExitStack() as ctx:
        wpool = ctx.enter_context(tc.tile_pool(name="wpool", bufs=1))
        qkpool = ctx.enter_context(tc.tile_pool(name="qkpool", bufs=1))
        plane = ctx.enter_context(tc.tile_pool(name="plane", bufs=1))
        stage = ctx.enter_context(tc.tile_pool(name="stage", bufs=2))
        hpool = ctx.enter_context(tc.tile_pool(name="hpool", bufs=2))
        spool = ctx.enter_context(tc.tile_pool(name="spool", bufs=4))
        opool = ctx.enter_context(tc.tile_pool(name="opool", bufs=2))
        ps_a = ctx.enter_context(tc.tile_pool(name="ps_a", bufs=2, space="PSUM"))
        ps_b = ctx.enter_context(tc.tile_pool(name="ps_b", bufs=2, space="PSUM"))

        # identity (bf16) for mask-add matmul + PE transposes
        ident_f = plane.tile([P, P], F32)
        make_identity(nc, ident_f)
        ident = plane.tile([P, P], BF16)
        nc.vector.tensor_copy(out=ident, in_=ident_f)

        # persistent planes (all [P, NQ, S] single tiles, one DMA each)
        masknegt = plane.tile([P, NQ, S], BF16, name="masknegt")
        bias2t = plane.tile([P, NQ, S], BF16, name="bias2t")
        nc.sync.dma_start(out=masknegt, in_=maskneg_d)
        maskneg = [masknegt[:, j, :] for j in range(NQ)]
        bias2 = [bias2t[:, j, :] for j in range(NQ)]

        # bias2' = (w1*exp(-dist) + w2*adj + cb)*iw0 + maskneg*iw0
        distf = stage.tile([P, NQ, S], BF16, tag="distf", name="distf")
        adjf = stage.tile([P, NQ, S], BF16, tag="adjf", name="adjf")
        nc.sync.dma_start(out=distf, in_=dist_d)
        nc.sync.dma_start(out=adjf, in_=adj_d)
        distk = stage.tile([P, NQ, S], BF16, tag="distk", name="distk")
        nc.scalar.activation(out=distk, in_=distf, func=Exp, scale=-1.0)
        t0 = stage.tile([P, NQ, S], BF16, tag="t0", name="t0")
        nc.scalar.activation(out=t0, in_=adjf, func=Copy,
                             bias=cb * iw0, scale=w2 * iw0)
        t1 = stage.tile([P, NQ, S], BF16, tag="t1", name="t1")
        nc.vector.scalar_tensor_tensor(
            out=t1, in0=distk, scalar=w1 * iw0, in1=t0,
            op0=ALU.mult, op1=ALU.add,
        )
        nc.vector.scalar_tensor_tensor(
            out=bias2t, in0=masknegt, scalar=iw0, in1=t1,
            op0=ALU.mult, op1=ALU.add,
        )

        # weights: one [P, ND, D] tile per tensor, DMA'd in 2-d-chunk pieces
        # (wf loaded later into the dead wq slot to fit SBUF)
        wt = {}
        for name in ("q", "k", "v"):
            wt[name] = wpool.tile([P, ND, D], BF16, tag=f"w{name}", name=f"w{name}")
        w_tiles = {n: [wt[n][:, d, :] for d in range(ND)] for n in ("q", "k", "v")}

        # QT/KT [D,S] as ND x [P,S];  V [S,D] as NQ x [P,D]
        QT = [qkpool.tile([P, S], BF16, tag=f"QT{_}", name=f"QT{_}") for _ in range(ND)]
        KT = [qkpool.tile([P, S], BF16, tag=f"KT{_}", name=f"KT{_}") for _ in range(ND)]
        V = [qkpool.tile([P, D], BF16, tag=f"V{_}", name=f"V{_}") for _ in range(NQ)]
        ctxT = [qkpool.tile([P, S], BF16, tag=f"ctxT{_}", name=f"ctxT{_}")
                for _ in range(ND)]

        xpool = ctx.enter_context(tc.tile_pool(name="xpool", bufs=1))
        xt = {}
        for name in ("q", "k", "v"):
            xt[name] = xpool.tile([P, ND, S], BF16, tag=f"x{name}", name=f"x{name}")
        xq = [xt["q"][:, d, :] for d in range(ND)]
        xk = [xt["k"][:, d, :] for d in range(ND)]
        xv = [xt["v"][:, d, :] for d in range(ND)]
        # interleaved per 2-d-chunk so the first proj matmuls start early
        for name, d_ap, x_ap in (("q", wqT_d, xqT_d), ("k", wkT_d, xkT_d),
                                 ("v", wvT_d, xvT_d)):
            for dd in range(0, ND, 2):
                nc.sync.dma_start(out=wt[name][:, dd:dd + 2, :],
                                  in_=d_ap[:, dd:dd + 2, :])
                nc.sync.dma_start(out=xt[name][:, dd:dd + 2, :],
                                  in_=x_ap[:, dd:dd + 2, :])

        _proj_T(nc, ps_b, w_tiles["q"], xq, QT, nc.scalar.copy)
        _proj_T(nc, ps_b, w_tiles["k"], xk, KT, nc.scalar.copy)
        # V[s,o]: psum[s-chunk, o-half] = xvT[d, s-slice]^T x wvT[d, o-half]
        for si in range(NQ):
            for oh in range(2):
                ps = ps_b.tile([P, S], F32, tag="proj", name=f"psv{si}{oh}")
                for d in range(ND):
                    nc.tensor.matmul(
                        ps, lhsT=xv[d][:, ts(si, P)],
                        rhs=w_tiles["v"][d][:, ts(oh, S)],
                        start=(d == 0), stop=(d == ND - 1),
                    )
                nc.vector.tensor_copy(out=V[si][:, ts(oh, S)], in_=ps)

        # fc weights into the (now dead) wq slot
        wt["f"] = wpool.tile([P, ND, D], BF16, tag="wq", name="wf")
        w_tiles["f"] = [wt["f"][:, d, :] for d in range(ND)]
        for dd in range(0, ND, 2):
            nc.sync.dma_start(out=wt["f"][:, dd:dd + 2, :],
                              in_=wfT_d[:, dd:dd + 2, :])

        # main loop, software-pipelined: head h's scores/exp1 (front) are
        # emitted before head h-1's softmax tail, so the PE always has the
        # next head's matmuls queued while the DVE/ACT chain of the previous
        # head completes (keeps the PE dense and HAM warm).
        def front(h):
            qh, qp = h // 2, (h % 2) * DK
            E1 = hpool.tile([P, NQ, S], BF16, tag="E1", bufs=3, name=f"E1_{h}")
            r1 = spool.tile([P, NQ], F32, tag="r1", name=f"r1_{h}")
            for j in range(NQ):
                ps = ps_a.tile([P, S], F32, tag="scores", bufs=5, name=f"pss{h}_{j}")
                nc.tensor.matmul(
                    ps, lhsT=QT[qh][qp:qp + DK, ts(j, P)],
                    rhs=KT[qh][qp:qp + DK, :], start=True, stop=False,
                )
                nc.tensor.matmul(
                    ps, lhsT=ident, rhs=maskneg[j], start=False, stop=True,
                )
                nc.scalar.activation(
                    out=E1[:, j, :], in_=ps, func=Exp,
                    accum_out=r1[:, j:j + 1],
                )
            return E1, r1

        pair_ps = {}

        def tail(h, E1, r1):
            qp = (h % 2) * DK
            if h % 2 == 0:
                pair_ps[h // 2] = ps_b.tile([P, S], F32, tag="ctx", bufs=1,
                                            name=f"psc{h // 2}")
            ps_ctx = pair_ps[h // 2]
            r1i = spool.tile([P, NQ], F32, tag="r1i", name=f"r1i_{h}")
            nc.vector.reciprocal(out=r1i, in_=r1)
            if not fold:
                r1iw = spool.tile([P, NQ], F32, tag="r1iw", name=f"r1iw_{h}")
                nc.vector.tensor_scalar_mul(r1iw, r1i, w0)
                r1i = r1iw

            tmid = hpool.tile([P, NQ, S], BF16, tag="tmid", name=f"tm_{h}")
            E2 = hpool.tile([P, NQ, S], BF16, tag="E2", name=f"E2_{h}")
            r2 = spool.tile([P, NQ], F32, tag="r2", name=f"r2_{h}")
            for j in range(NQ):
                nc.vector.scalar_tensor_tensor(
                    out=tmid[:, j, :], in0=E1[:, j, :],
                    scalar=r1i[:, j:j + 1], in1=bias2[j],
                    op0=ALU.mult, op1=ALU.add,
                )
                nc.scalar.activation(
                    out=E2[:, j, :], in_=tmid[:, j, :], func=Exp,
                    scale=exp2_scale, accum_out=r2[:, j:j + 1],
                )
            r2i = spool.tile([P, NQ], F32, tag="r2i", name=f"r2i_{h}")
            nc.vector.reciprocal(out=r2i, in_=r2)

            attn = hpool.tile([P, NQ, S], BF16, tag="attn", name=f"at_{h}")
            for j in range(NQ):
                nc.vector.tensor_scalar_mul(
                    attn[:, j, :], E2[:, j, :], r2i[:, j:j + 1],
                )
            nc.sync.dma_start(out=attn_d[h], in_=attn)

            attnT = hpool.tile([P, NQ, S], BF16, tag="attnT", name=f"aT_{h}")
            for c in range(NQ):
                for j in range(NQ):
                    nc.sync.dma_start_transpose(
                        out=attnT[:, c, ts(j, P)], in_=attn[:, j, ts(c, P)],
                    )
            for c in range(NQ):
                nc.tensor.matmul(
                    ps_ctx[qp:qp + DK, :], lhsT=V[c][:, h * DK:(h + 1) * DK],
                    rhs=attnT[:, c, :], start=(c == 0), stop=(c == NQ - 1),
                    tile_position=(0, qp),
                )
            if h % 2 == 1:
                nc.vector.tensor_copy(out=ctxT[h // 2], in_=ps_ctx)

        pending = None
        for h in range(H):
            cur = front(h)
            if pending is not None:
                tail(h - 1, *pending)
            pending = cur
        tail(H - 1, *pending)

        # fc: outputT[o,s] = sum_d wfT[d, o-slice] x ctxT[d]
        for o in range(ND):
            ps = ps_b.tile([P, S], F32, tag="proj", name=f"psf{o}")
            for d in range(ND):
                nc.tensor.matmul(
                    ps, lhsT=w_tiles["f"][d][:, ts(o, P)], rhs=ctxT[d],
                    start=(d == 0), stop=(d == ND - 1),
                )
            outt = opool.tile([P, S], F32, tag="outT", name=f"outt{o}")
            nc.vector.tensor_copy(out=outt, in_=ps)
            nc.sync.dma_start(out=outT_d[o], in_=outt)

    nc.compile()
    return nc


def kernel(input_Q, input_K, input_V, attn_mask, adj_matrix, dist_matrix,
           W_Q, W_K, W_V, W_fc, conv_w, conv_b):
    global LAST_RESULTS
    bf = ml_dtypes.bfloat16
    w0, w1, w2 = (float(conv_w[0]), float(conv_w[1]), float(conv_w[2]))
    cb = float(conv_b[0])

    nc = _build_program(w0, w1, w2, cb)

    def ileave(a2d):
        # [n*128, c] -> [128, n, c] with t[p, d, c] = a2d[d*128+p, c]
        n = a2d.shape[0] // P
        return np.ascontiguousarray(a2d.reshape(n, P, -1).transpose(1, 0, 2))

    wqT = ileave(np.asarray(W_Q, np.float32).T / 8.0).astype(bf)
    wkT = ileave(np.asarray(W_K, np.float32).T).astype(bf)
    wvT = ileave(np.asarray(W_V, np.float32).T).astype(bf)
    wfT = ileave(np.asarray(W_fc, np.float32).T).astype(bf)

    in_maps = []
    for b in range(B):
        maskneg = (np.asarray(attn_mask[b], np.float32)
                   * np.float32(-1e9))
        in_maps.append({
            "xqT": ileave(np.asarray(input_Q[b], np.float32).T).astype(bf),
            "xkT": ileave(np.asarray(input_K[b], np.float32).T).astype(bf),
            "xvT": ileave(np.asarray(input_V[b], np.float32).T).astype(bf),
            "wqT": wqT, "wkT": wkT, "wvT": wvT, "wfT": wfT,
            "maskneg": ileave(maskneg).astype(bf),
            "adj": ileave(np.asarray(adj_matrix[b], np.float32)).astype(bf),
            "dist": ileave(np.asarray(dist_matrix[b], np.float32)).astype(bf),
        })

    if PROFILE:
        _install_profile_hook()
    res = run_bass_kernel_spmd(
        nc, in_maps, core_ids=list(range(B)), trace=PROFILE, **TRACE_KWARGS,
    )
    LAST_RESULTS = res

    output = np.stack([
        np.concatenate([res.results[b][f"outT{o}"] for o in range(ND)], axis=0)
        .T.astype(np.float32)
        for b in range(B)
    ])
    def dileave(a3d):
        # [128, n, c] -> [n*128, c]
        p, n, c = a3d.shape
        return a3d.transpose(1, 0, 2).reshape(n * p, c)

    attn = np.stack([
        np.stack([dileave(res.results[b][f"attn{h}"]).astype(np.float32)
                  for h in range(H)])
        for b in range(B)
    ])
    return (output, attn)


# revision 32
# speedup vs baseline: 2.3886x; 2.3886x over previous
"""Trainium2 Bass kernel for nn_MultiHeadAttention_73212012528234.

Sharding: data-parallel over batch — core c handles batch b=c (B=8, 8 cores).

Per-core pipeline (all matmuls bf16 on PE, fp32 PSUM accumulation):
  1. QT/KT = (W/8 or W) @ X^T  -> [D, S] head-major layout (d on partitions)
     V = X @ W_V^T             -> [S, D] natural layout
  2. per head h: scores psum = QT_h-slice x KT_h  (+ identity-matmul adds
     -1e9*mask);  E1 = exp(psum) with ACT accum -> rowsum r1
  3. tmid = (E1 * 1/r1) + bias2'  (DVE scalar_tensor_tensor);
     E2 = exp(w0 * tmid) with accum -> r2;  attn = E2 * 1/r2 -> HBM
     (bf16, host casts to f32)
  4. PE-transpose attn -> attnT;  contextT_h = V_h-slice^T x attnT
  5. outputT = W_fc^T-chunks x contextT -> HBM (f32, host transposes)

bias2' = (w1*exp(-dist) + w2*adj + cb - 1e9*mask)/w0 built on device; the /w0
fold lets exp2's ACT scale apply w0 with zero extra ops.
"""

from contextlib import ExitStack

import ml_dtypes
import numpy as np

import concourse.bass as bass
import concourse.tile as tile
from concourse import bacc, mybir
from concourse.bass import ts
from concourse.bass_utils import run_bass_kernel_spmd
from concourse.masks import make_identity

B, S, D, H, DK = 8, 512, 1024, 16, 64
P = 128
NQ = S // P  # 4 q/k chunks of 128
ND = D // P  # 8 d chunks of 128
BF16 = mybir.dt.bfloat16
F32 = mybir.dt.float32
Exp = mybir.ActivationFunctionType.Exp
Copy = mybir.ActivationFunctionType.Copy
ALU = mybir.AluOpType

# toggles (test.py may flip before calling kernel)
PROFILE = False
TRACE_KWARGS = {}
LAST_RESULTS = None


def _install_profile_hook():
    """The agent image's antenv lacks axon_hooks; synthesize it so
    run_bass_kernel_spmd(trace=True) can capture NTFF profiles."""
    import sys, types
    try:
        from antenv.axon_hooks import get_axon_ntff_profile_hook  # noqa
        return
    except ImportError:
        pass
    import antenv
    from trn_agent_boot.trn_boot import _ntff_profile_via_ctypes
    mod = types.ModuleType("antenv.axon_hooks")
    mod._hook = _ntff_profile_via_ctypes("/opt/axon/libaxon_pjrt.so")

    def set_axon_ntff_profile_hook(h):
        mod._hook = h

    def get_axon_ntff_profile_hook():
        return mod._hook

    mod.set_axon_ntff_profile_hook = set_axon_ntff_profile_hook
    mod.get_axon_ntff_profile_hook = get_axon_ntff_profile_hook
    sys.modules["antenv.axon_hooks"] = mod
    antenv.axon_hooks = mod


def _proj_T(nc, psum, w_tiles, xT_tiles, out_tiles, copy_engine):
    """out[o,s] = sum_d w[d,o-slice] * xT[d,s]  -> out_tiles: ND x [P, S]."""
    for o in range(ND):
        ps = psum.tile([P, S], F32, tag="proj", name=f"ps_{out_tiles[o].name}")
        for d in range(ND):
            nc.tensor.matmul(
                ps, lhsT=w_tiles[d][:, ts(o, P)], rhs=xT_tiles[d],
                start=(d == 0), stop=(d == ND - 1),
            )
        copy_engine(out=out_tiles[o], in_=ps)


def _build_program(w0, w1, w2, cb):
    fold = abs(w0) >= 1e-4
    iw0 = (1.0 / w0) if fold else 1.0
    exp2_scale = w0 if fold else 1.0

    nc = bacc.Bacc("TRN2", target_bir_lowering=False, debug=False)

    # inputs arrive partition-interleaved: t[p, d, c] = orig[d*128+p, c]
    xqT_d = nc.dram_tensor("xqT", [P, ND, S], BF16, kind="ExternalInput").ap()
    xkT_d = nc.dram_tensor("xkT", [P, ND, S], BF16, kind="ExternalInput").ap()
    xvT_d = nc.dram_tensor("xvT", [P, ND, S], BF16, kind="ExternalInput").ap()
    wqT_d = nc.dram_tensor("wqT", [P, ND, D], BF16, kind="ExternalInput").ap()
    wkT_d = nc.dram_tensor("wkT", [P, ND, D], BF16, kind="ExternalInput").ap()
    wvT_d = nc.dram_tensor("wvT", [P, ND, D], BF16, kind="ExternalInput").ap()
    wfT_d = nc.dram_tensor("wfT", [P, ND, D], BF16, kind="ExternalInput").ap()
    maskneg_d = nc.dram_tensor("maskneg", [P, NQ, S], BF16, kind="ExternalInput").ap()
    adj_d = nc.dram_tensor("adj", [P, NQ, S], BF16, kind="ExternalInput").ap()
    dist_d = nc.dram_tensor("dist", [P, NQ, S], BF16, kind="ExternalInput").ap()

    attn_d = [nc.dram_tensor(f"attn{h}", [P, NQ, S], BF16, kind="ExternalOutput").ap()
              for h in range(H)]
    outT_d = [nc.dram_tensor(f"outT{o}", [P, S], F32, kind="ExternalOutput").ap()
              for o in range(ND)]

    with tile.TileContext(nc) as tc, ExitStack() as ctx:
        wpool = ctx.enter_context(tc.tile_pool(name="wpool", bufs=1))
        qkpool = ctx.enter_context(tc.tile_pool(name="qkpool", bufs=1))
        plane = ctx.enter_context(tc.tile_pool(name="plane", bufs=1))
        stage = ctx.enter_context(tc.tile_pool(name="stage", bufs=2))
        hpool = ctx.enter_context(tc.tile_pool(name="hpool", bufs=2))
        spool = ctx.enter_context(tc.tile_pool(name="spool", bufs=4))
        opool = ctx.enter_context(tc.tile_pool(name="opool", bufs=2))
        ps_a = ctx.enter_context(tc.tile_pool(name="ps_a", bufs=2, space="PSUM"))
        ps_b = ctx.enter_context(tc.tile_pool(name="ps_b", bufs=2, space="PSUM"))

        # identity (bf16) for mask-add matmul + PE transposes
        ident_f = plane.tile([P, P], F32)
        make_identity(nc, ident_f)
        ident = plane.tile([P, P], BF16)
        nc.vector.tensor_copy(out=ident, in_=ident_f)

        # persistent planes (all [P, NQ, S] single tiles, one DMA each)
        masknegt = plane.tile([P, NQ, S], BF16, name="masknegt")
        bias2t = plane.tile([P, NQ, S], BF16, name="bias2t")
        nc.sync.dma_start(out=masknegt, in_=maskneg_d)
        maskneg = [masknegt[:, j, :] for j in range(NQ)]
        bias2 = [bias2t[:, j, :] for j in range(NQ)]

        # bias2' = (w1*exp(-dist) + w2*adj + cb)*iw0 + maskneg*iw0
        distf = stage.tile([P, NQ, S], BF16, tag="distf", name="distf")
        adjf = stage.tile([P, NQ, S], BF16, tag="adjf", name="adjf")
        nc.sync.dma_start(out=distf, in_=dist_d)
        nc.sync.dma_start(out=adjf, in_=adj_d)
        distk = stage.tile([P, NQ, S], BF16, tag="distk", name="distk")
        nc.scalar.activation(out=distk, in_=distf, func=Exp, scale=-1.0)
        t0 = stage.tile([P, NQ, S], BF16, tag="t0", name="t0")
        nc.scalar.activation(out=t0, in_=adjf, func=Copy,
                             bias=cb * iw0, scale=w2 * iw0)
        t1 = stage.tile([P, NQ, S], BF16, tag="t1", name="t1")
        nc.vector.scalar_tensor_tensor(
            out=t1, in0=distk, scalar=w1 * iw0, in1=t0,
            op0=ALU.mult, op1=ALU.add,
        )
        nc.vector.scalar_tensor_tensor(
            out=bias2t, in0=masknegt, scalar=iw0, in1=t1,
            op0=ALU.mult, op1=ALU.add,
        )

        # weights: one [P, ND, D] tile per tensor, DMA'd in 2-d-chunk pieces
        # (wf loaded later into the dead wq slot to fit SBUF)
        wt = {}
        for name in ("q", "k", "v"):
            wt[name] = wpool.tile([P, ND, D], BF16, tag=f"w{name}", name=f"w{name}")
        w_tiles = {n: [wt[n][:, d, :] for d in range(ND)] for n in ("q", "k", "v")}

        # QT/KT [D,S] as ND x [P,S];  V [S,D] as NQ x [P,D]
        QT = [qkpool.tile([P, S], BF16, tag=f"QT{_}", name=f"QT{_}") for _ in range(ND)]
        KT = [qkpool.tile([P, S], BF16, tag=f"KT{_}", name=f"KT{_}") for _ in range(ND)]
        V = [qkpool.tile([P, D], BF16, tag=f"V{_}", name=f"V{_}") for _ in range(NQ)]
        ctxT = [qkpool.tile([P, S], BF16, tag=f"ctxT{_}", name=f"ctxT{_}")
                for _ in range(ND)]

        xpool = ctx.enter_context(tc.tile_pool(name="xpool", bufs=1))
        xt = {}
        for name in ("q", "k", "v"):
            xt[name] = xpool.tile([P, ND, S], BF16, tag=f"x{name}", name=f"x{name}")
        xq = [xt["q"][:, d, :] for d in range(ND)]
        xk = [xt["k"][:, d, :] for d in range(ND)]
        xv = [xt["v"][:, d, :] for d in range(ND)]
        # interleaved per 2-d-chunk so the first proj matmuls start early
        for name, d_ap, x_ap in (("q", wqT_d, xqT_d), ("k", wkT_d, xkT_d),
                                 ("v", wvT_d, xvT_d)):
            for dd in range(0, ND, 2):
                nc.sync.dma_start(out=wt[name][:, dd:dd + 2, :],
                                  in_=d_ap[:, dd:dd + 2, :])
                nc.sync.dma_start(out=xt[name][:, dd:dd + 2, :],
                                  in_=x_ap[:, dd:dd + 2, :])

        _proj_T(nc, ps_b, w_tiles["q"], xq, QT, nc.scalar.copy)
        _proj_T(nc, ps_b, w_tiles["k"], xk, KT, nc.scalar.copy)
        # V[s,o]: psum[s-chunk, o-half] = xvT[d, s-slice]^T x wvT[d, o-half]
        for si in range(NQ):
            for oh in range(2):
                ps = ps_b.tile([P, S], F32, tag="proj", name=f"psv{si}{oh}")
                for d in range(ND):
                    nc.tensor.matmul(
                        ps, lhsT=xv[d][:, ts(si, P)],
                        rhs=w_tiles["v"][d][:, ts(oh, S)],
                        start=(d == 0), stop=(d == ND - 1),
                    )
                nc.vector.tensor_copy(out=V[si][:, ts(oh, S)], in_=ps)

        # fc weights into the (now dead) wq slot
        wt["f"] = wpool.tile([P, ND, D], BF16, tag="wq", name="wf")
        w_tiles["f"] = [wt["f"][:, d, :] for d in range(ND)]
        for dd in range(0, ND, 2):
            nc.sync.dma_start(out=wt["f"][:, dd:dd + 2, :],
                              in_=wfT_d[:, dd:dd + 2, :])

        # main loop, software-pipelined: head h's scores/exp1 (front) are
        # emitted before head h-1's softmax tail, so the PE always has the
        # next head's matmuls queued while the DVE/ACT chain of the previous
        # head completes (keeps the PE dense and HAM warm).
        def front(h):
            qh, qp = h // 2, (h % 2) * DK
            E1 = hpool.tile([P, NQ, S], BF16, tag="E1", bufs=3, name=f"E1_{h}")
            r1 = spool.tile([P, NQ], F32, tag="r1", name=f"r1_{h}")
            for j in range(NQ):
                ps = ps_a.tile([P, S], F32, tag="scores", bufs=3, name=f"pss{h}_{j}")
                nc.tensor.matmul(
                    ps, lhsT=QT[qh][qp:qp + DK, ts(j, P)],
                    rhs=KT[qh][qp:qp + DK, :], start=True, stop=False,
                )
                nc.tensor.matmul(
                    ps, lhsT=ident, rhs=maskneg[j], start=False, stop=True,
                )
                nc.scalar.activation(
                    out=E1[:, j, :], in_=ps, func=Exp,
                    accum_out=r1[:, j:j + 1],
                )
            return E1, r1

        pair_ps = {}

        def tail(h, E1, r1):
            qp = (h % 2) * DK
            if h % 2 == 0:
                pair_ps[h // 2] = ps_b.tile([P, S], F32, tag="ctx", bufs=1,
                                            name=f"psc{h // 2}")
            ps_ctx = pair_ps[h // 2]
            r1i = spool.tile([P, NQ], F32, tag="r1i", name=f"r1i_{h}")
            nc.vector.reciprocal(out=r1i, in_=r1)
            if not fold:
                r1iw = spool.tile([P, NQ], F32, tag="r1iw", name=f"r1iw_{h}")
                nc.vector.tensor_scalar_mul(r1iw, r1i, w0)
                r1i = r1iw

            tmid = hpool.tile([P, NQ, S], BF16, tag="tmid", name=f"tm_{h}")
            E2 = hpool.tile([P, NQ, S], BF16, tag="E2", name=f"E2_{h}")
            r2 = spool.tile([P, NQ], F32, tag="r2", name=f"r2_{h}")
            for j in range(NQ):
                nc.vector.scalar_tensor_tensor(
                    out=tmid[:, j, :], in0=E1[:, j, :],
                    scalar=r1i[:, j:j + 1], in1=bias2[j],
                    op0=ALU.mult, op1=ALU.add,
                )
                nc.scalar.activation(
                    out=E2[:, j, :], in_=tmid[:, j, :], func=Exp,
                    scale=exp2_scale, accum_out=r2[:, j:j + 1],
                )
            r2i = spool.tile([P, NQ], F32, tag="r2i", name=f"r2i_{h}")
            nc.vector.reciprocal(out=r2i, in_=r2)

            attn = hpool.tile([P, NQ, S], BF16, tag="attn", name=f"at_{h}")
            for j in range(NQ):
                nc.vector.tensor_scalar_mul(
                    attn[:, j, :], E2[:, j, :], r2i[:, j:j + 1],
                )
            nc.sync.dma_start(out=attn_d[h], in_=attn)

            attnT = hpool.tile([P, NQ, S], BF16, tag="attnT", name=f"aT_{h}")
            for c in range(NQ):
                ps_t = ps_a.tile([P, S], BF16, tag="transp", name=f"pst{h}_{c}")
                for j in range(NQ):
                    nc.tensor.matmul(
                        ps_t[:, ts(j, P)], lhsT=attn[:, j, ts(c, P)],
                        rhs=ident, is_transpose=True,
                        start=(j == 0), stop=(j == NQ - 1),
                    )
                nc.vector.tensor_copy(out=attnT[:, c, :], in_=ps_t)
            for c in range(NQ):
                nc.tensor.matmul(
                    ps_ctx[qp:qp + DK, :], lhsT=V[c][:, h * DK:(h + 1) * DK],
                    rhs=attnT[:, c, :], start=(c == 0), stop=(c == NQ - 1),
                    tile_position=(0, qp),
                )
            if h % 2 == 1:
                nc.vector.tensor_copy(out=ctxT[h // 2], in_=ps_ctx)

        pending = None
        for h in range(H):
            cur = front(h)
            if pending is not None:
                tail(h - 1, *pending)
            pending = cur
        tail(H - 1, *pending)

        # fc: outputT[o,s] = sum_d wfT[d, o-slice] x ctxT[d]
        for o in range(ND):
            ps = ps_b.tile([P, S], F32, tag="proj", name=f"psf{o}")
            for d in range(ND):
                nc.tensor.matmul(
                    ps, lhsT=w_tiles["f"][d][:, ts(o, P)], rhs=ctxT[d],
                    start=(d == 0), stop=(d == ND - 1),
                )
            outt = opool.tile([P, S], F32, tag="outT", name=f"outt{o}")
            nc.vector.tensor_copy(out=outt, in_=ps)
            nc.sync.dma_start(out=outT_d[o], in_=outt)

    nc.compile()
    return nc


def kernel(input_Q, input_K, input_V, attn_mask, adj_matrix, dist_matrix,
           W_Q, W_K, W_V, W_fc, conv_w, conv_b):
    global LAST_RESULTS
    bf = ml_dtypes.bfloat16
    w0, w1, w2 = (float(conv_w[0]), float(conv_w[1]), float(conv_w[2]))
    cb = float(conv_b[0])

    nc = _build_program(w0, w1, w2, cb)

    def ileave(a2d):
        # [n*128, c] -> [128, n, c] with t[p, d, c] = a2d[d*128+p, c]
        n = a2d.shape[0] // P
        return np.ascontiguousarray(a2d.reshape(n, P, -1).transpose(1, 0, 2))

    wqT = ileave(np.asarray(W_Q, np.float32).T / 8.0).astype(bf)
    wkT = ileave(np.asarray(W_K, np.float32).T).astype(bf)
    wvT = ileave(np.asarray(W_V, np.float32).T).astype(bf)
    wfT = ileave(np.asarray(W_fc, np.float32).T).astype(bf)

    in_maps = []
    for b in range(B):
        maskneg = (np.asarray(attn_mask[b], np.float32)
                   * np.float32(-1e9))
        in_maps.append({
            "xqT": ileave(np.asarray(input_Q[b], np.float32).T).astype(bf),
            "xkT": ileave(np.asarray(input_K[b], np.float32).T).astype(bf),
            "xvT": ileave(np.asarray(input_V[b], np.float32).T).astype(bf),
            "wqT": wqT, "wkT": wkT, "wvT": wvT, "wfT": wfT,
            "maskneg": ileave(maskneg).astype(bf),
            "adj": ileave(np.asarray(adj_matrix[b], np.float32)).astype(bf),
            "dist": ileave(np.asarray(dist_matrix[b], np.float32)).astype(bf),
        })

    if PROFILE:
        _install_profile_hook()
    res = run_bass_kernel_spmd(
        nc, in_maps, core_ids=list(range(B)), trace=PROFILE, **TRACE_KWARGS,
    )
    LAST_RESULTS = res

    output = np.stack([
        np.concatenate([res.results[b][f"outT{o}"] for o in range(ND)], axis=0)
        .T.astype(np.float32)
        for b in range(B)
    ])
    def dileave(a3d):
        # [128, n, c] -> [n*128, c]
        p, n, c = a3d.shape
        return a3d.transpose(1, 0, 2).reshape(n * p, c)

    attn = np.stack([
        np.stack([dileave(res.results[b][f"attn{h}"]).astype(np.float32)
                  for h in range(H)])
        for b in range(B)
    ])
    return (output, attn)
